# revision 12
# baseline (speedup 1.0000x reference)
"""FAGCN message-passing layer on 8 Trainium2 NeuronCores (Bass/Tile).

Strategy (v3: dst-sharded, degree-sorted dst-per-partition windows,
bulk dma_gather over an int16-range-split node table):
  - Nodes are 1D-partitioned across 8 cores by dst (12544/core). Each
    core's dst nodes are sorted by in-degree and packed into 98 windows
    of 128; window w partition p owns one dst node, so slot counts per
    partition track the window's (near-uniform) degree.
  - Per-core node TABLE: Haug[row] = [h*d (64 f16), gs, gs, pad] in
    256B rows (dma_gather granularity), with rows ordered per core and
    split into 4 ranges of <=32768 rows so gather indices fit int16.
    A greedy balanced coloring assigns each referenced src node to a
    range so that every dst node's edges split evenly across ranges
    (minimizes per-window slot padding). All table compute (h*d, h@W)
    runs on device; the host only chooses row order / indices.
  - Main loop: for each pair of windows, one dma_gather per range
    fetches all needed src rows (int16 indices, 256B elems); ACT
    computes th=tanh(gs+gd_dst) with gd as per-partition bias; DVE
    multiplies messages into the gathered tile's pad columns and
    reduces over the slot axis; z row = d_dst * sum. z is written
    partition-major and un-permuted on the host.
"""
import numpy as np

P = 128
D = 64
EL = 128          # table row: h' (64) + gs (65) + gs dup + pad = 256B
N_CORES = 8
NPC = 12544
NW = NPC // P     # 98
N_NODES_MAX = 100352
R2 = 102400       # padded table rows (50 * 2048)
NRANGE = 4
RSTART = [0, 32768, 65536, 98304]
RCAP = [32767, 32767, 32767, 4095]   # last row of each range = zero row
ZROWR = [32767, 32767, 32767, 4095]  # in-range index of the zero row
HL_ROWS = 13312   # permuted local-h rows padded to 104 windows
BB_A = 16         # build-A nodes per tile-row (50 iters of 2048 rows)
GRP = 2           # windows per gather group (49 groups)
GZ = 14           # windows per z flush


def _color_ranges(src_e, dl_e, npc):
    """Greedy balanced range coloring: assign each referenced src node a
    range 0..2 (overflow 3) minimizing per-dst edge imbalance."""
    order_e = np.argsort(src_e, kind="stable")
    ss = src_e[order_e]
    dd = dl_e[order_e]
    uniq, starts = np.unique(ss, return_index=True)
    ends = np.append(starts[1:], ss.size)
    refcnt = ends - starts
    # process srcs by refcount desc (high-impact first)
    proc = np.argsort(-refcnt, kind="stable")
    color = np.full(N_NODES_MAX, 3, np.int8)
    cnt = np.zeros((npc, 3), np.int32)
    fill = [0, 0, 0]
    for k in proc:
        s = uniq[k]
        dsts = dd[starts[k]:ends[k]]
        score = cnt[dsts, :].sum(axis=0)
        for g in np.argsort(score, kind="stable"):
            if fill[g] < RCAP[g]:
                break
        else:
            g = 3
        color[s] = g
        if g < 3:
            fill[g] += 1
            np.add.at(cnt, (dsts, g), 1)
    return color, uniq, refcnt


def _idx_layout(NTWG):
    """Group-major idx column offsets: blocks ordered (group, g, w) so each
    gather group's indices are one contiguous DMA and each (group, g)
    gather's indices are contiguous."""
    icol_off = np.zeros((NW, NRANGE), np.int64)
    c = 0
    for w0 in range(0, NW, GRP):
        ws = range(w0, min(w0 + GRP, NW))
        for g in range(NRANGE):
            for w in ws:
                icol_off[w, g] = c
                c += int(NTWG[w, g]) * 8
    return icol_off, c


def _host_prep(h, d, gate_W, gate_b, edge_src, edge_dst):
    """Shard + layout preparation (pure data movement / indexing)."""
    N = h.shape[0]
    h_pad = np.zeros((R2, D), dtype=np.float32)
    h_pad[:N] = np.asarray(h, dtype=np.float32)
    d_pad = np.zeros((R2,), dtype=np.float32)
    d_pad[:N] = np.asarray(d, dtype=np.float32)

    WSRC = np.tile(np.asarray(gate_W[0, D:2 * D], np.float32), (P, 1))
    WDST = np.tile(np.asarray(gate_W[0, 0:D], np.float32), (P, 1))
    BREP = np.full((P, 1), float(np.asarray(gate_b).reshape(-1)[0]), np.float32)

    order = np.argsort(edge_dst, kind="stable")
    sd = np.asarray(edge_dst)[order].astype(np.int64)
    ss = np.asarray(edge_src)[order].astype(np.int64)
    bounds = np.searchsorted(sd, np.arange(N_CORES + 1) * NPC)

    cores = []
    for c in range(N_CORES):
        lo, hi = int(bounds[c]), int(bounds[c + 1])
        dl = sd[lo:hi] - c * NPC
        src = ss[lo:hi]

        color, uniq, refcnt = _color_ranges(src, dl, NPC)

        # table row assignment: per range, referenced srcs by refcount desc
        tau = np.full(N_NODES_MAX, -1, np.int64)
        g_all = np.full(N_NODES_MAX, -1, np.int8)
        g_all[uniq] = color[uniq]
        used = np.zeros(NRANGE, np.int64)
        rc_full = np.zeros(N_NODES_MAX, np.int64)
        rc_full[uniq] = refcnt
        for g in range(NRANGE):
            nodes_g = uniq[color[uniq] == g]
            nodes_g = nodes_g[np.argsort(-rc_full[nodes_g], kind="stable")]
            assert nodes_g.size <= RCAP[g], (g, nodes_g.size)
            tau[nodes_g] = RSTART[g] + np.arange(nodes_g.size)
            used[g] = nodes_g.size
        # unreferenced nodes: stuff anywhere with free rows (never gathered)
        unref = np.where(g_all < 0)[0]
        pos = 0
        for g in range(NRANGE):
            free = RCAP[g] - used[g]
            take = min(free, unref.size - pos)
            if take > 0:
                tau[unref[pos:pos + take]] = RSTART[g] + used[g] + np.arange(take)
                used[g] += take
                pos += take
        assert pos == unref.size

        # per-edge range + rank within (dst, range)
        ge = g_all[src].astype(np.int64)
        key = dl * NRANGE + ge
        c_g = np.bincount(key, minlength=NPC * NRANGE).reshape(NPC, NRANGE)
        deg = c_g.sum(axis=1)
        kstart = np.zeros(NPC * NRANGE, np.int64)
        kstart[1:] = np.cumsum(np.bincount(key, minlength=NPC * NRANGE))[:-1]
        ord2 = np.argsort(key, kind="stable")
        trank = np.empty(src.size, np.int64)
        trank[ord2] = np.arange(src.size) - kstart[key[ord2]]

        perm = np.argsort(-deg, kind="stable")       # window packing
        rankof = np.empty(NPC, np.int64)
        rankof[perm] = np.arange(NPC)

        ntwg_c = c_g[perm].reshape(NW, P, NRANGE).max(axis=1)  # [NW, 4]
        cores.append(dict(
            dl=dl, src=src, tau=tau, ge=ge, trank=trank, perm=perm,
            rankof=rankof, ntwg=ntwg_c,
        ))

    NTWG = np.maximum.reduce([cc["ntwg"] for cc in cores])     # [NW, 4]
    key = tuple(int(x) for x in NTWG.reshape(-1))
    icol_off, TOTI = _idx_layout(NTWG)

    in_maps = []
    for c in range(N_CORES):
        cc = cores[c]
        tau, ge, trank, perm, rankof = (cc["tau"], cc["ge"], cc["trank"],
                                        cc["perm"], cc["rankof"])
        dl, src = cc["dl"], cc["src"]

        r = rankof[dl]
        p_arr = r % P
        w_arr = r // P

        # flat slot index within (w,g): i = trank*128 + p
        idxw = np.zeros((16, TOTI), np.int16)
        # initialize pads to the per-range zero row
        for w in range(NW):
            for g in range(NRANGE):
                nt = int(NTWG[w, g])
                if nt == 0:
                    continue
                o = int(icol_off[w, g])
                idxw[:, o:o + nt * 8] = ZROWR[g]
        sel_i = trank * P + p_arr                     # position within block
        col16 = sel_i // 16
        row16 = sel_i % 16
        val = (tau[src] - np.asarray(RSTART, np.int64)[ge]).astype(np.int64)
        dest_col = icol_off[w_arr, ge] + col16
        idxw[row16, dest_col] = val.astype(np.int16)
        IDXW = np.tile(idxw, (8, 1))                  # replicate per 16-group

        # permuted local h for gd (2-row-packed build-B layout)
        perm_pad = np.full(HL_ROWS, c * NPC, np.int64)
        perm_pad[:NPC] = perm + c * NPC
        rr = np.arange(HL_ROWS)
        jj, rem = rr // 256, rr % 256
        pp_, two = rem // 2, rem % 2
        hloc2 = h_pad[perm_pad[(2 * jj + two) * P + pp_]]

        dcol = d_pad[perm + c * NPC].reshape(NW, P).T.copy()

        # per-core table-ordered h and d (4-row-packed build-A layout)
        node_of_row = np.full(R2, -1, np.int64)
        node_of_row[tau[tau >= 0]] = np.where(tau >= 0)[0]
        h_tab = np.zeros((R2, D), np.float32)
        d_tab = np.zeros((R2,), np.float32)
        m = node_of_row >= 0
        h_tab[m] = h_pad[node_of_row[m]]
        d_tab[m] = d_pad[node_of_row[m]]
        DROWS4 = d_tab.reshape(R2 // 512, 128, 4).transpose(1, 0, 2).reshape(P, R2 // P)
        DROWS4 = np.ascontiguousarray(DROWS4)

        in_maps.append({
            "h_tab": h_tab, "drows4": DROWS4, "wsrc": WSRC, "wdst": WDST,
            "brep": BREP, "idxw": np.ascontiguousarray(IDXW),
            "hloc2": hloc2, "dcol": dcol,
            "_perm": perm,
        })
    return in_maps, key


def _build_program(key):
    import concourse.bacc as bacc
    import concourse.tile as tile
    from concourse import bass, mybir

    NTWG = np.asarray(key, np.int64).reshape(NW, NRANGE)
    icol_off, TOTI = _idx_layout(NTWG)
    NTW = NTWG.sum(axis=1)                 # slots per window
    S2MAX = int(max(NTW[w0:w0 + GRP].sum() for w0 in range(0, NW, GRP)))

    f32, f16 = mybir.dt.float32, mybir.dt.float16
    i16 = mybir.dt.int16

    nc = bacc.Bacc("TRN2", target_bir_lowering=False, debug=False,
                   num_devices=N_CORES)
    htab_d = nc.dram_tensor("h_tab", [R2, D], f32, kind="ExternalInput")
    drows4_d = nc.dram_tensor("drows4", [P, R2 // P], f32, kind="ExternalInput")
    wsrc_d = nc.dram_tensor("wsrc", [P, D], f32, kind="ExternalInput")
    wdst_d = nc.dram_tensor("wdst", [P, D], f32, kind="ExternalInput")
    brep_d = nc.dram_tensor("brep", [P, 1], f32, kind="ExternalInput")
    idxw_d = nc.dram_tensor("idxw", [P, TOTI], i16, kind="ExternalInput")
    hloc2_d = nc.dram_tensor("hloc2", [HL_ROWS, D], f32, kind="ExternalInput")
    dcol_d = nc.dram_tensor("dcol", [P, NW], f32, kind="ExternalInput")
    z_d = nc.dram_tensor("z", [P, NW, D], f32, kind="ExternalOutput")

    haug_i = nc.dram_tensor("haug_i", [R2, EL], f16, kind="Internal")

    with tile.TileContext(nc) as tc:
        with tc.tile_pool(name="const", bufs=1) as cp:
            dcol_t = cp.tile([P, NW], f32)
            nc.sync.dma_start(out=dcol_t[:], in_=dcol_d[:, :])
            gdw_t = cp.tile([P, HL_ROWS // P], f32)

            # ---------- build phase (pool closes -> SBUF freed) ----------
            with tc.tile_pool(name="bld", bufs=2) as bp:
                wsrc_t = cp.tile([P, D], f32)
                nc.sync.dma_start(out=wsrc_t[:], in_=wsrc_d[:, :])
                wdst_t = cp.tile([P, D], f32)
                nc.sync.dma_start(out=wdst_t[:], in_=wdst_d[:, :])
                brep_t = cp.tile([P, 1], f32)
                nc.sync.dma_start(out=brep_t[:], in_=brep_d[:, :])
                drows4_t = cp.tile([P, R2 // P], f32)
                nc.sync.dma_start(out=drows4_t[:], in_=drows4_d[:, :])

                # build A: table rows [h*d (64), gs, gs] from h_tab
                for s in range(R2 // (BB_A * P)):
                    rows = slice(s * BB_A * P, (s + 1) * BB_A * P)
                    h8 = bp.tile([P, 4, 4 * D], f32, tag="h8")
                    nc.sync.dma_start(
                        out=h8[:],
                        in_=htab_d[rows, :].rearrange(
                            "(j p four) e -> p j (four e)", p=P, four=4))
                    h8v = h8[:].rearrange("p j (f e) -> p j f e", e=D)
                    hp16 = bp.tile([P, 4, 4, EL], f16, tag="hp16")
                    ddv = drows4_t[:, s * BB_A:(s + 1) * BB_A].rearrange(
                        "p (j f a) -> p j f a", f=4, a=1)
                    nc.vector.tensor_tensor(
                        out=hp16[:, :, :, 0:D], in0=h8v,
                        in1=ddv.to_broadcast([P, 4, 4, D]),
                        op=mybir.AluOpType.mult)
                    prod = bp.tile([P, 4, 4, D], f32, tag="prod")
                    nc.vector.tensor_tensor(
                        out=prod[:], in0=h8v,
                        in1=wsrc_t[:].rearrange(
                            "p (a b e) -> p a b e", a=1, b=1
                        ).to_broadcast([P, 4, 4, D]),
                        op=mybir.AluOpType.mult)
                    gsb = bp.tile([P, 4, 4], f32, tag="gsb")
                    nc.vector.tensor_reduce(out=gsb[:], in_=prod[:],
                                            op=mybir.AluOpType.add,
                                            axis=mybir.AxisListType.X)
                    # gs (+bias) broadcast over cols 64:128 (fills row pad so
                    # the 512B-chunk DMA below reads fully-written SBUF)
                    nc.vector.tensor_scalar(
                        out=hp16[:, :, :, D:EL],
                        in0=gsb[:].rearrange("p j (f a) -> p j f a",
                                             a=1).to_broadcast([P, 4, 4, EL - D]),
                        scalar1=brep_t[:, 0:1], scalar2=None,
                        op0=mybir.AluOpType.add)
                    nc.sync.dma_start(
                        out=haug_i[rows, :].rearrange(
                            "(j p four) e -> p j (four e)", p=P, four=4),
                        in_=hp16[:].rearrange("p j f e -> p j (f e)"))

                # build B: gd for local (window-permuted) nodes
                for s in range(HL_ROWS // (8 * P)):
                    rows = slice(s * 8 * P, (s + 1) * 8 * P)
                    hl8 = bp.tile([P, 4, 2 * D], f32, tag="h8")
                    nc.sync.dma_start(
                        out=hl8[:],
                        in_=hloc2_d[rows, :].rearrange(
                            "(j p two) e -> p j (two e)", p=P, two=2))
                    prodb = bp.tile([P, 4, 2, D], f32, tag="prodb")
                    nc.vector.tensor_tensor(
                        out=prodb[:],
                        in0=hl8[:].rearrange("p j (t e) -> p j t e", e=D),
                        in1=wdst_t[:].rearrange(
                            "p (a b e) -> p a b e", a=1, b=1
                        ).to_broadcast([P, 4, 2, D]),
                        op=mybir.AluOpType.mult)
                    nc.vector.tensor_reduce(
                        out=gdw_t[:, s * 8:(s + 1) * 8].rearrange(
                            "p (j two) -> p j two", two=2),
                        in_=prodb[:], op=mybir.AluOpType.add,
                        axis=mybir.AxisListType.X)

            # ---------- main loop: GRP windows per gather group ----------
            with tc.tile_pool(name="main", bufs=2) as mp:
                IMAX = S2MAX * 8
                zbuf = None
                for w0 in range(0, NW, GRP):
                    ws = list(range(w0, min(w0 + GRP, NW)))
                    # slot layout in ga mirrors the idx layout: (g, w)-major
                    toff = {}
                    t = 0
                    for g in range(NRANGE):
                        for w in ws:
                            toff[(w, g)] = t
                            t += int(NTWG[w, g])
                    S2 = t
                    grp_base = int(icol_off[ws[0], 0])
                    ga = mp.tile([P, S2MAX, EL], f16, tag="ga")
                    idx_t = mp.tile([P, IMAX], i16, tag="idx")
                    nc.sync.dma_start(
                        out=idx_t[:, 0:S2 * 8],
                        in_=idxw_d[:, grp_base:grp_base + S2 * 8])
                    # gathers per (group, range), chunked to <=1024 indices
                    # (SWDGE descriptor-ring capacity)
                    for g in range(NRANGE):
                        nt_tot = sum(int(NTWG[w, g]) for w in ws)
                        if nt_tot == 0:
                            continue
                        o0 = toff[(next(w for w in ws if NTWG[w, g] > 0), g)]
                        for co in range(0, nt_tot, 8):
                            o = o0 + co
                            nt = min(8, nt_tot - co)
                            nc.gpsimd.dma_gather(
                                out_ap=ga[:, o:o + nt, :],
                                in_ap=haug_i[RSTART[g]:
                                             RSTART[g] + RCAP[g] + 1, :],
                                idxs_ap=idx_t[:, o * 8:(o + nt) * 8],
                                num_idxs=nt * P,
                                num_idxs_reg=nt * P,
                                elem_size=EL)
                    # per-window gate + message + reduce
                    th = mp.tile([P, S2MAX], f16, tag="th")
                    for w in ws:
                        if w % GZ == 0:
                            zbuf = mp.tile([P, GZ, D], f32, tag="zbuf")
                        first = True
                        zacc = mp.tile([P, D], f32, tag="zacc")
                        ztmp = mp.tile([P, D], f32, tag="ztmp")
                        for g in range(NRANGE):
                            nt = int(NTWG[w, g])
                            if nt == 0:
                                continue
                            o = toff[(w, g)]
                            nc.scalar.activation(
                                out=th[:, o:o + nt], in_=ga[:, o:o + nt, D],
                                func=mybir.ActivationFunctionType.Tanh,
                                bias=gdw_t[:, w:w + 1])
                            nc.vector.tensor_tensor(
                                out=ga[:, o:o + nt, D:D + D],
                                in0=ga[:, o:o + nt, 0:D],
                                in1=th[:, o:o + nt].rearrange(
                                    "p (t a) -> p t a", a=1
                                ).to_broadcast([P, nt, D]),
                                op=mybir.AluOpType.mult)
                            red_out = zacc if first else ztmp
                            nc.vector.tensor_reduce(
                                out=red_out[:],
                                in_=ga[:, o:o + nt, D:D + D].rearrange(
                                    "p t d -> p d t"),
                                op=mybir.AluOpType.add,
                                axis=mybir.AxisListType.X)
                            if not first:
                                nc.vector.tensor_tensor(
                                    out=zacc[:], in0=zacc[:], in1=ztmp[:],
                                    op=mybir.AluOpType.add)
                            first = False
                        if first:  # empty window
                            nc.vector.memset(zbuf[:, w % GZ, :], 0.0)
                        else:
                            nc.vector.tensor_scalar_mul(
                                zbuf[:, w % GZ, :], zacc[:],
                                dcol_t[:, w:w + 1])
                        if w % GZ == GZ - 1:
                            nc.sync.dma_start(
                                out=z_d[:, w - GZ + 1:w + 1, :], in_=zbuf[:])

    nc.compile()
    return nc


_CACHE = {}


def kernel(h, d, gate_W, gate_b, edge_src, edge_dst):
    from concourse.bass_utils import run_bass_kernel_spmd

    N = h.shape[0]
    in_maps, key = _host_prep(h, d, gate_W, gate_b, edge_src, edge_dst)
    if key not in _CACHE:
        _CACHE[key] = _build_program(key)
    nc = _CACHE[key]
    perms = [m.pop("_perm") for m in in_maps]
    res = run_bass_kernel_spmd(nc, in_maps, core_ids=list(range(N_CORES)))
    z = np.empty((N_CORES * NPC, D), np.float32)
    for c in range(N_CORES):
        zc = res.results[c]["z"]                       # [128, NW, 64]
        zperm = zc.transpose(1, 0, 2).reshape(NPC, D)  # rank-major
        z[perms[c] + c * NPC] = zperm
    return np.ascontiguousarray(z[:N]).astype(np.float32)
